# revision 19
# baseline (speedup 1.0000x reference)
"""Trainium2 Bass kernel for nn_ClassAwareLoss (class-aware frame loss).

Contract: kernel(**inputs) takes the FULL unsharded inputs (numpy arrays,
keyed as in setup_inputs()) and returns the FULL output (a float32 scalar).

Strategy (data-parallel over batch, per the sharding hint), v2:
  The reference computes a dense [B, 1600] dots matrix, but per sample only
  the frames of its own class (<= 31 of 1600) carry nonzero weight.  We sort
  samples by class on the host (a pure permutation -- the loss is a sum over
  samples), so each 128-sample tile spans <= 2..3 classes and needs only a
  small per-tile frame block (W columns, W = max span rounded up, 64 for the
  reference distribution).  This cuts PE work ~8x and ScalarE work ~25x.

  Per core (2048 sorted samples = 16 tiles of 128):
    - One DoubleRow matmul per tile computes psum = x_tile @ [x_tile | Fb]^T:
      cols 0:128 hold the Gram matrix (diagonal = squared norms), cols
      128:128+W hold raw dots against the tile's frame block.
    - diag extract (DVE/GpSimd alternating): sq_b = Gram[b, b]
    - norm = sqrt(sq) (ScalarE), g = 1/norm (DVE), reg = sum (norm-1)^2
    - S = (g*dots - 1)^2 via one ScalarE activation (Square, scale=g, bias=-1)
    - cal_b += sum_j Wt[b, j] * S[b, j] (Wt = cosine_c[target]*classmask,
      host-built) via stt with accum, DVE/GpSimd alternating.
  Host sums the per-partition partials in float64.
"""

import sys
import types
from contextlib import ExitStack

sys.path.insert(0, "/opt/trn_rl_repo")

import numpy as np
import ml_dtypes

# ---------------------------------------------------------------------------
# antenv.axon_hooks shim: lets run_bass_kernel_spmd(trace=True) capture NTFF
# profiles under axon.  Harmless when BASS_TRACE is not set.
# ---------------------------------------------------------------------------
try:
    import antenv

    if "antenv.axon_hooks" not in sys.modules:
        _mod = types.ModuleType("antenv.axon_hooks")
        _hook = [None]
        _mod.set_axon_ntff_profile_hook = lambda h: _hook.__setitem__(0, h)
        _mod.get_axon_ntff_profile_hook = lambda: _hook[0]
        sys.modules["antenv.axon_hooks"] = _mod
        antenv.axon_hooks = _mod
        try:
            from trn_agent_boot.trn_boot import _ntff_profile_via_ctypes

            _mod.set_axon_ntff_profile_hook(
                _ntff_profile_via_ctypes("/opt/axon/libaxon_pjrt.so")
            )
        except Exception:
            pass
except Exception:
    pass

import concourse.bass as bass
import concourse.tile as tile
import concourse.bass_utils as bass_utils
from concourse import bacc, mybir

# No cloud bucket in this container; keep artifacts local.
bass_utils.upload_artifacts = lambda tmpdir: "local://" + tmpdir

# ---------------------------------------------------------------------------
# Problem constants (input-independent)
# ---------------------------------------------------------------------------
N_CORES = 8

BF16 = mybir.dt.bfloat16
FP8 = mybir.dt.float8e4
F32 = mybir.dt.float32
AF = mybir.ActivationFunctionType
ALU = mybir.AluOpType

_COMPILED = {}      # (NT, W) -> compiled Bacc
LAST_RESULT = None  # BassKernelResults of the most recent run (for test.py)


def _group_sizes(NT):
    """Uneven norm groups: big groups early (ScalarE batching), tiny tail
    group so the final diag->sqrt->recip->S chain is short."""
    sizes, rem = [], NT
    while rem > 0:
        sizes.append(min(4, rem))
        rem -= sizes[-1]
    return sizes


def _build_program(NT, W):
    """SPMD program: NT sample-tiles of 128 per core, W frame cols per tile."""
    CPT = 2 * (128 + W)          # block cols per tile (two D-halves)
    GS = _group_sizes(NT)        # e.g. [6, 6, 3, 1] for NT=16
    GMAX = max(GS)
    CW = GMAX * W                # cal psum region width

    nc = bacc.Bacc(
        "TRN2", target_bir_lowering=False, debug=False, num_devices=N_CORES
    )

    blocks_in = nc.dram_tensor("blocks", [128, NT * CPT], FP8, kind="ExternalInput").ap()
    wt_in = nc.dram_tensor("wt", [128, NT * W], BF16, kind="ExternalInput").ap()
    id_in = nc.dram_tensor("ident", [128, 128], BF16, kind="ExternalInput").ap()
    out_c = nc.dram_tensor("outc", [1, CW + NT], F32, kind="ExternalOutput").ap()

    with tile.TileContext(nc) as tc:
        with ExitStack() as ctx:
            const_pool = ctx.enter_context(tc.tile_pool(name="const", bufs=1))
            rot_pool = ctx.enter_context(tc.tile_pool(name="rot", bufs=3))
            psum_pool = ctx.enter_context(
                tc.tile_pool(name="psum", bufs=7, space="PSUM")
            )
            psum_cal = ctx.enter_context(
                tc.tile_pool(name="psumc", bufs=1, space="PSUM")
            )
            prod_pool = ctx.enter_context(tc.tile_pool(name="prodp", bufs=4))

            ones_sb = const_pool.tile([128, 1], BF16, tag="ones")
            nc.vector.memset(ones_sb[:], 1.0)
            neg_one = const_pool.tile([128, 1], F32, tag="negone")
            nc.vector.memset(neg_one[:], -1.0)
            # prefetch both ScalarE activation tables during the DMA fill
            dummy = const_pool.tile([128, 1], F32, tag="dumy")
            nc.scalar.activation(dummy[:], neg_one[:], AF.Sqrt, scale=-1.0)
            nc.scalar.activation(dummy[:], neg_one[:], AF.Square)

            ident_sb = const_pool.tile([128, 128], BF16, tag="ident")
            nc.gpsimd.dma_start(ident_sb[:], id_in[:])
            wt_sb = const_pool.tile([128, NT * W], BF16, tag="wt")
            nc.gpsimd.dma_start(wt_sb[:], wt_in[:])

            blocks_sb = const_pool.tile([128, NT * CPT], FP8, tag="blocks")
            chunks = [2, 4, 4, NT - 10] if NT > 10 else GS
            acc = 0
            for ch in chunks:
                lo, hi = acc * CPT, (acc + ch) * CPT
                nc.sync.dma_start(blocks_sb[:, lo:hi], blocks_in[:, lo:hi])
                acc += ch

            sq_all = const_pool.tile([128, NT], F32, tag="sq")
            norm_all = const_pool.tile([128, NT], F32, tag="nm")
            g_all = const_pool.tile([128, NT], F32, tag="g")
            cal_ps = psum_cal.tile([1, CW + NT], F32, tag="calps")

            prods = []
            jlo = 0
            for grp, gw in enumerate(GS):
                jhi = jlo + gw
                psums = []
                for j in range(jlo, jhi):
                    blk = blocks_sb[:, j * CPT : (j + 1) * CPT].rearrange(
                        "p (two n) -> p two n", two=2
                    )
                    ps = psum_pool.tile([128, 128 + W], F32, tag="ps")
                    nc.tensor.matmul(
                        ps[:],
                        lhsT=blk[:, :, 0:128],
                        rhs=blk,
                        start=True,
                        stop=True,
                        perf_mode=mybir.MatmulPerfMode.DoubleRow,
                    )
                    psums.append(ps)
                    # diagonal of the Gram block -> squared norms (DVE only:
                    # GpSimd cannot read PSUM)
                    dump = rot_pool.tile([128, 128], BF16, tag="dd")
                    nc.vector.scalar_tensor_tensor(
                        out=dump[:],
                        in0=ident_sb[:],
                        scalar=1.0,
                        in1=ps[:, 0:128],
                        op0=ALU.mult,
                        op1=ALU.mult,
                        accum_out=sq_all[:, j : j + 1],
                    )

                # frames were host-scaled by 16 (fp8 range); psum dots = 16*d.
                # norm16 = sqrt(256*sq) = 16*norm, g16 = 1/(16*norm) undoes it.
                gsl = slice(jlo, jhi)
                nc.scalar.activation(norm_all[:, gsl], sq_all[:, gsl], AF.Sqrt, scale=256.0)
                nc.vector.reciprocal(g_all[:, gsl], norm_all[:, gsl])

                if jhi == NT:
                    # reg = sum_b (norm - 1)^2: emit before this group's S
                    # pass so it overlaps the ScalarE backlog
                    regdump = rot_pool.tile([128, NT], BF16, tag="rd")
                    nc.scalar.activation(
                        regdump[:],
                        norm_all[:],
                        AF.Square,
                        scale=0.0625,
                        bias=neg_one[:],
                    )
                    nc.tensor.matmul(
                        cal_ps[:, CW : CW + NT],
                        lhsT=ones_sb[:],
                        rhs=regdump[:],
                        start=True,
                        stop=True,
                        skip_group_check=True,
                    )

                s_q = rot_pool.tile([128, gw * W], BF16, tag=f"s{gw}")
                for j in range(jlo, jhi):
                    nc.scalar.activation(
                        s_q[:, (j - jlo) * W : (j - jlo + 1) * W],
                        psums[j - jlo][:, 128 : 128 + W],
                        AF.Square,
                        bias=neg_one[:],
                        scale=g_all[:, j : j + 1],
                    )
                prod = prod_pool.tile([128, gw * W], BF16, tag=f"prod{grp}")
                nc.gpsimd.tensor_tensor(
                    out=prod[:], in0=s_q[:],
                    in1=wt_sb[:, jlo * W : jhi * W], op=ALU.mult,
                )
                prods.append(prod)
                jlo = jhi

            # column-sums of Wt*S accumulate on the PE, after every big
            # matmul so the in-order PE stream never stalls mid-window
            for grp, prod in enumerate(prods):
                pw = GS[grp] * W
                nc.tensor.matmul(
                    cal_ps[:, 0:pw],
                    lhsT=ones_sb[:],
                    rhs=prod[:],
                    start=(grp == 0),
                    stop=(grp == len(prods) - 1),
                    skip_group_check=True,
                )

            cal_sb = const_pool.tile([1, CW + NT], F32, tag="calsb")
            nc.vector.tensor_copy(cal_sb[:], cal_ps[:])
            nc.sync.dma_start(out_c[:], cal_sb[:])

    nc.compile()
    return nc


def _prepare(inputs):
    """Host-side layout prep: sort by class, build per-tile blocks/weights."""
    x = np.asarray(inputs["input"], dtype=np.float32)            # [B, D]
    frames = np.asarray(inputs["frames"], dtype=np.float32)      # [F, D]
    cosine_c = np.asarray(inputs["cosine_c"], dtype=np.float32)  # [nc]
    target = np.asarray(inputs["target"]).astype(np.int64)       # [B]
    frame_class = np.asarray(inputs["frame_class"]).astype(np.int64)  # [F]

    B, D = x.shape
    assert D == 256 and B % (N_CORES * 128) == 0
    NT = B // (N_CORES * 128)

    perm = np.argsort(target, kind="stable")
    xs = x[perm].astype(ml_dtypes.float8_e4m3)
    ts = target[perm]
    fr_bf = (frames * 16.0).astype(ml_dtypes.float8_e4m3)

    # per-class frame row indices
    ncls = int(cosine_c.shape[0])
    cls_rows = [np.where(frame_class == c)[0] for c in range(ncls)]

    n_tiles = B // 128
    tile_fidx = []
    maxspan = 1
    for t in range(n_tiles):
        cls = np.unique(ts[t * 128 : (t + 1) * 128])
        fidx = np.concatenate([cls_rows[c] for c in cls])
        tile_fidx.append(fidx)
        maxspan = max(maxspan, len(fidx))
    W = max(32, -(-maxspan // 32) * 32)
    assert W <= 128, f"frame span {maxspan} too large for single-matmul layout"
    CPT = 2 * (128 + W)

    cw = cosine_c[ts]  # [B] per-sample cosine weight (sorted order)

    in_maps = []
    for c in range(N_CORES):
        blocks = np.zeros((128, NT * CPT), dtype=ml_dtypes.float8_e4m3)
        wt = np.zeros((128, NT * W), dtype=ml_dtypes.bfloat16)
        for jj in range(NT):
            t = c * NT + jj
            sl = slice(t * 128, (t + 1) * 128)
            xt = xs[sl]                      # [128, 256] bf16
            fidx = tile_fidx[t]
            base = jj * CPT
            for h in range(2):
                hb = base + h * (128 + W)
                blocks[:, hb : hb + 128] = xt[:, h * 128 : (h + 1) * 128].T
                fb = fr_bf[fidx][:, h * 128 : (h + 1) * 128]  # [nf, 128]
                blocks[:, hb + 128 : hb + 128 + len(fidx)] = fb.T
            mask = frame_class[fidx][None, :] == ts[sl][:, None]  # [128, nf]
            wt[:, jj * W : jj * W + len(fidx)] = (
                cw[sl][:, None] * mask
            ).astype(ml_dtypes.bfloat16)
        in_maps.append(
            {
                "blocks": blocks,
                "wt": wt,
                "ident": np.eye(128, dtype=ml_dtypes.bfloat16),
            }
        )
    return in_maps, NT, W


def kernel(**inputs):
    global LAST_RESULT
    in_maps, NT, W = _prepare(inputs)
    key = (NT, W)
    if key not in _COMPILED:
        _COMPILED[key] = _build_program(NT, W)
    nc = _COMPILED[key]

    res = bass_utils.run_bass_kernel_spmd(
        nc, in_maps, core_ids=list(range(N_CORES))
    )
    LAST_RESULT = res

    B = NT * 128 * N_CORES
    CW = max(_group_sizes(NT)) * W
    caloss = 0.0
    reg = 0.0
    for c in range(N_CORES):
        o = res.results[c]["outc"].astype(np.float64)
        caloss += o[0, 0:CW].sum()
        reg += o[0, CW : CW + NT].sum()
    val = (caloss + 0.0006 * reg) / B
    return np.float32(val)
